# revision 1
# baseline (speedup 1.0000x reference)
"""Longformer sliding-window + global attention layer on 8 Trainium2 NeuronCores.

Sharding: sequence-parallel over the 4096 tokens (512 per core, all 12 heads).
Each core recomputes the k/v halo (256 tokens each side) and the 64 global
k/v tokens locally from zero-padded hsT input, so the program is uniform SPMD.
The global-query rows (first 64 tokens attend to everything) are computed as
flash-style partial sums over each core's 512 tokens and combined with an
on-device AllReduce; every core finalizes the identical 64 global rows.

Layout strategy (all matmuls float32r = TF32, 1 cycle/row at N>=256):
  - hsT [hidden, tokens] feeds projections in both orientations.
  - Banded attention uses transposed scores: scoresT[j, t] = kT.T-slice @ qT,
    exp (scale=1/8 folded in), 0/1 mask multiply, then PV with the natural-v
    tiles carrying an appended ones-column -> the softmax denominator falls out
    of the same accumulation. A PE transpose + per-partition reciprocal scale
    normalizes [t, d] output tiles.
"""
import numpy as np

import concourse.bacc as bacc
import concourse.mybir as mybir
import concourse.tile as tile
from concourse.bass_utils import run_bass_kernel_spmd

F32 = mybir.dt.float32
F32R = mybir.dt.float32r
Exp = mybir.ActivationFunctionType.Exp

S, H, NH, HD = 4096, 768, 12, 64
C = 256               # chunk / one-sided window
G = 64                # global tokens
NCORE = 8
TPC = S // NCORE      # 512 tokens per core
EXT = TPC + 2 * C     # 1024 ext window
COLS = EXT + G        # 1088 = ext | glob
KC = H // 128         # 6 hidden chunks
VW = 66               # per-head v block: 64 v | ones | pad
SCALE = 1.0 / 8.0     # 1/sqrt(HD)

# j-tiles that need the 0/1 mask multiply, per local chunk
MASKED = {0: (0, 1, 2, 4, 5), 1: (0, 1, 4, 5)}

_PROG_CACHE = {}


def _build_program(with_bias: bool):
    nc = bacc.Bacc("TRN2", target_bir_lowering=False, debug=False,
                   num_devices=NCORE)
    d_hsT = nc.declare_dram_parameter("hsT", [H, COLS], F32R, isOutput=False)
    d_w = {
        n: nc.declare_dram_parameter(n, [H, H], F32R, isOutput=False)
        for n in ("wq", "wk", "wv", "wkg", "wvg", "wqg")
    }
    d_masks = nc.declare_dram_parameter("masks", [2, KC, 128, C], F32R,
                                        isOutput=False)
    d_consts = nc.declare_dram_parameter("consts", [128, 152], F32R,
                                         isOutput=False)
    if with_bias:
        d_brow = nc.declare_dram_parameter("biasrow", [7, COLS], F32R,
                                           isOutput=False)
    d_out = nc.declare_dram_parameter("out", [TPC, H], F32, isOutput=True)
    d_outg = nc.declare_dram_parameter("outg", [G, H], F32, isOutput=True)

    with tile.TileContext(nc) as tc:
        with (
            tc.tile_pool(name="const", bufs=1) as const,
            tc.tile_pool(name="wfull", bufs=2) as wfull,
            tc.tile_pool(name="work", bufs=4) as work,
            tc.tile_pool(name="w2", bufs=3) as w2,
            tc.tile_pool(name="late", bufs=1) as late,
            tc.tile_pool(name="dram", bufs=2, space="DRAM") as dram,
            tc.tile_pool(name="psA", bufs=2, space="PSUM") as psA,
            tc.tile_pool(name="psS", bufs=2, space="PSUM") as psS,
            tc.tile_pool(name="psO", bufs=2, space="PSUM") as psO,
            tc.tile_pool(name="psT", bufs=2, space="PSUM") as psT,
        ):
            # ---- resident loads ----
            csb = const.tile([128, 152], F32R)
            nc.sync.dma_start(out=csb, in_=d_consts[:])
            ident = csb[:, 0:128]
            vpat = csb[:, 128:152]          # [128, 12*2] ones/zeros pattern

            hsb = const.tile([128, KC, COLS], F32R)
            nc.sync.dma_start(out=hsb,
                              in_=d_hsT.rearrange("(kc p) t -> p kc t", p=128))
            msb = const.tile([128, 2, KC, C], F32R)
            nc.sync.dma_start(out=msb,
                              in_=d_masks.rearrange("c j p t -> p c j t"))
            if with_bias:
                bsb = const.tile([7, COLS], F32R)
                nc.sync.dma_start(out=bsb, in_=d_brow[:])

            kT = const.tile([128, KC, COLS], F32R)    # [o, t] all heads
            qT = const.tile([128, KC, TPC], F32R)
            vE = const.tile([128, 9, NH * VW], F32R)  # natural v + ones cols
            kgT = const.tile([128, KC, TPC], F32R)
            vgN = const.tile([128, 4, NH * VW], F32R)
            qgT = const.tile([128, KC, G], F32R)
            qgn = const.tile([G, H], F32R)

            def load_w(name):
                t = wfull.tile([128, KC, H], F32R, tag="wfull")
                nc.sync.dma_start(
                    out=t, in_=d_w[name].rearrange("(kc p) o -> p kc o", p=128))
                return t

            def proj_T(dst, wsb, segs, bias_idx, dst_off):
                # dst[o, t] = W.T @ hsT cols
                for oc in range(KC):
                    for c0, cn in segs:
                        ps = psA.tile([128, 512], F32, tag="psA")
                        for kc in range(KC):
                            nc.tensor.matmul(
                                out=ps[:, 0:cn],
                                lhsT=wsb[:, kc, oc * 128:(oc + 1) * 128],
                                rhs=hsb[:, kc, c0:c0 + cn],
                                start=(kc == 0),
                                stop=(kc == KC - 1 and not with_bias),
                            )
                        if with_bias:
                            nc.tensor.matmul(
                                out=ps[:, 0:cn],
                                lhsT=bsb[1 + bias_idx:2 + bias_idx,
                                         oc * 128:(oc + 1) * 128],
                                rhs=bsb[0:1, 0:cn],
                                start=False, stop=True,
                            )
                        nc.vector.tensor_copy(
                            out=dst[:, oc, c0 - dst_off:c0 - dst_off + cn],
                            in_=ps[:, 0:cn])

            def proj_nat(dst, wsb, tts, tok_off, bias_idx):
                # dst[t, head-block] with 66-stride head blocks
                for ti, tt in enumerate(tts):
                    # ones/pad pattern for cols 64/65 of each 66-block
                    nc.sync.dma_start(
                        out=dst[:, ti, :].rearrange("p (h x) -> p h x", x=VW)[:, :, 64:66],
                        in_=vpat.rearrange("p (h x) -> p h x", x=2))
                    tok0 = tok_off + tt * 128
                    rows = 128 if tok0 + 128 <= COLS else COLS - tok0
                    for o0, on in ((0, 512), (512, 256)):
                        ps = psA.tile([128, 512], F32, tag="psA")
                        for kc in range(KC):
                            nc.tensor.matmul(
                                out=ps[:rows, 0:on],
                                lhsT=hsb[:, kc, tok0:tok0 + rows],
                                rhs=wsb[:, kc, o0:o0 + on],
                                start=(kc == 0),
                                stop=(kc == KC - 1 and not with_bias),
                            )
                        if with_bias:
                            nc.tensor.matmul(
                                out=ps[:rows, 0:on],
                                lhsT=bsb[0:1, 0:rows],
                                rhs=bsb[1 + bias_idx:2 + bias_idx, o0:o0 + on],
                                start=False, stop=True,
                            )
                        nc.vector.tensor_copy(
                            out=dst[:rows, ti, :].rearrange(
                                "p (h x) -> p h x", x=VW)[:, o0 // 64:(o0 + on) // 64, 0:64],
                            in_=ps[:rows, 0:on].rearrange("p (h x) -> p h x", x=64))

            # ---- projections feeding the global-row partials first ----
            w = load_w("wkg")
            proj_T(kgT, w, ((C, 512),), 3, C)
            w = load_w("wvg")
            proj_nat(vgN, w, (2, 3, 4, 5), 0, 4)
            w = load_w("wqg")
            # qg natural [G, H] then PE-transpose into qgT
            for o0, on in ((0, 512), (512, 256)):
                ps = psA.tile([128, 512], F32, tag="psA")
                for kc in range(KC):
                    nc.tensor.matmul(
                        out=ps[0:G, 0:on],
                        lhsT=hsb[:, kc, EXT:EXT + G],
                        rhs=w[:, kc, o0:o0 + on],
                        start=(kc == 0), stop=(kc == KC - 1 and not with_bias))
                if with_bias:
                    nc.tensor.matmul(
                        out=ps[0:G, 0:on], lhsT=bsb[0:1, 0:G],
                        rhs=bsb[6:7, o0:o0 + on], start=False, stop=True)
                nc.vector.tensor_copy(out=qgn[:, o0:o0 + on], in_=ps[0:G, 0:on])
            for oc in range(KC):
                pstr = psT.tile([128, G], F32R, tag="psT")
                nc.tensor.transpose(pstr, qgn[0:G, oc * 128:(oc + 1) * 128],
                                    ident[0:G, 0:G])
                nc.vector.tensor_copy(out=qgT[:, oc, :], in_=pstr)

            # ---- global-row partial attention over own 512 tokens ----
            partial = dram.tile([G, NH * VW], F32)
            reduced = dram.tile([G, NH * VW], F32)
            for h in range(NH):
                dd = 64 * (h % 2)
                pc = h // 2
                psg = psA.tile([G, 512], F32, tag="psA")
                nc.tensor.matmul(out=psg, lhsT=qgT[dd:dd + 64, pc, :],
                                 rhs=kgT[dd:dd + 64, pc, :],
                                 start=True, stop=True)
                pg = work.tile([G, 512], F32R, tag="pg")
                nc.scalar.activation(out=pg, in_=psg, func=Exp, scale=SCALE)
                roff = min(VW * h, NH * VW - 288)
                boff = VW * h - roff
                pspv = psA.tile([G, 288], F32, tag="psA")
                for kt in range(4):
                    pstr = psT.tile([128, G], F32R, tag="psT")
                    nc.tensor.transpose(pstr, pg[:, kt * 128:(kt + 1) * 128],
                                        ident[0:G, 0:G])
                    pgt = w2.tile([128, G], F32R, tag="pgt")
                    nc.vector.tensor_copy(out=pgt, in_=pstr)
                    nc.tensor.matmul(out=pspv, lhsT=pgt,
                                     rhs=vgN[:, kt, roff:roff + 288],
                                     start=(kt == 0), stop=(kt == 3))
                part = w2.tile([G, VW], F32, tag="part")
                nc.vector.tensor_copy(out=part, in_=pspv[:, boff:boff + VW])
                nc.sync.dma_start(out=partial[:, h * VW:(h + 1) * VW], in_=part)

            nc.gpsimd.collective_compute(
                "AllReduce", mybir.AluOpType.add,
                replica_groups=[list(range(NCORE))],
                ins=[partial.opt()], outs=[reduced.opt()])

            # ---- banded + global-column attention (the bulk) ----
            w = load_w("wq")
            proj_T(qT, w, ((C, 512),), 0, C)
            w = load_w("wk")
            proj_T(kT, w, ((0, 512), (512, 320), (832, 256)), 1, 0)
            w = load_w("wv")
            proj_nat(vE, w, (0, 1, 2, 3, 4, 5, 6, 7, 8), 0, 2)

            osb = late.tile([128, 4, H], F32, tag="osb")
            for h in range(NH):
                dd = 64 * (h % 2)
                pc = h // 2
                for cl in range(2):
                    pso = psO.tile([VW, C], F32, tag="psO")
                    for jt in range(KC):
                        pss = psS.tile([128, C], F32, tag="psS")
                        nc.tensor.matmul(
                            out=pss,
                            lhsT=kT[dd:dd + 64, pc,
                                    C * cl + 128 * jt:C * cl + 128 * (jt + 1)],
                            rhs=qT[dd:dd + 64, pc, C * cl:C * (cl + 1)],
                            start=True, stop=True)
                        ex = work.tile([128, C], F32R, tag="ex")
                        nc.scalar.activation(out=ex, in_=pss, func=Exp,
                                             scale=SCALE)
                        if jt in MASKED[cl]:
                            nc.gpsimd.tensor_mul(ex, ex, msb[:, cl, jt, :])
                        nc.tensor.matmul(
                            out=pso, lhsT=vE[:, 2 * cl + jt, VW * h:VW * (h + 1)],
                            rhs=ex, start=(jt == 0), stop=False)
                    # global-key columns, joint softmax
                    pss = psS.tile([128, C], F32, tag="psS")
                    nc.tensor.matmul(
                        out=pss[0:G, :], lhsT=kT[dd:dd + 64, pc, EXT:EXT + G],
                        rhs=qT[dd:dd + 64, pc, C * cl:C * (cl + 1)],
                        start=True, stop=True)
                    exg = work.tile([G, C], F32R, tag="exg")
                    nc.scalar.activation(out=exg, in_=pss[0:G, :], func=Exp,
                                         scale=SCALE)
                    nc.tensor.matmul(
                        out=pso, lhsT=vE[0:G, 8, VW * h:VW * (h + 1)],
                        rhs=exg, start=False, stop=True)
                    ot = w2.tile([VW, C], F32R, tag="ot")
                    nc.vector.tensor_copy(out=ot, in_=pso)
                    for tt in range(2):
                        pstr = psT.tile([128, VW], F32R, tag="psT")
                        nc.tensor.transpose(pstr, ot[:, tt * 128:(tt + 1) * 128],
                                            ident[0:VW, 0:VW])
                        rec = work.tile([128, 1], F32, tag="rec")
                        nc.vector.reciprocal(out=rec, in_=pstr[:, 64:65])
                        nc.vector.tensor_scalar_mul(
                            osb[:, 2 * cl + tt, 64 * h:64 * (h + 1)],
                            in0=pstr[:, 0:64], scalar1=rec)
            for i4 in range(4):
                nc.sync.dma_start(out=d_out[128 * i4:128 * (i4 + 1), :],
                                  in_=osb[:, i4, :])

            # ---- finalize global rows from the AllReduced partials ----
            red = late.tile([G, NH * VW], F32, tag="red")
            nc.sync.dma_start(out=red, in_=reduced)
            ogsb = late.tile([G, H], F32, tag="ogsb")
            for h in range(NH):
                rec = work.tile([G, 1], F32, tag="recg")
                nc.vector.reciprocal(out=rec, in_=red[:, h * VW + 64:h * VW + 65])
                nc.vector.tensor_scalar_mul(
                    ogsb[:, h * 64:(h + 1) * 64],
                    in0=red[:, h * VW:h * VW + 64], scalar1=rec)
            nc.sync.dma_start(out=d_outg[:], in_=ogsb)

    nc.compile()
    return nc


def _host_inputs(hs, weights, biases):
    """Build the 8 per-core input maps from full inputs."""
    hsT = np.ascontiguousarray(hs.T)               # [H, S]
    ident = np.eye(128, dtype=np.float32)
    vpat = np.zeros((128, 24), np.float32)
    vpat[:, 0::2] = 1.0
    consts = np.concatenate([ident, vpat], axis=1)  # [128, 152]

    with_bias = any(np.any(b) for b in biases)
    if with_bias:
        brow = np.zeros((7, COLS), np.float32)
        brow[0, :] = 1.0
        for i, b in enumerate(biases):
            brow[1 + i, :H] = b
    # masks per core
    jj = np.arange(3 * C)[:, None]                  # [768, 1] strip pos
    ii = np.arange(C)[None, :]                      # [1, 256] query in chunk
    in_maps = []
    for core in range(NCORE):
        hst = np.zeros((H, COLS), np.float32)
        lo = TPC * core - C
        hi = TPC * core + TPC + C
        clo, chi = max(lo, 0), min(hi, S)
        hst[:, clo - lo:chi - lo] = hsT[:, clo:chi]
        hst[:, EXT:] = hsT[:, :G]
        mk = np.zeros((2, KC, 128, C), np.float32)
        for cl in range(2):
            n = 2 * core + cl
            ka = n * C - C + jj
            valid = (jj >= ii) & (jj <= ii + 2 * C) & (ka >= G) & (ka < S)
            mk[cl] = valid.astype(np.float32).reshape(KC, 128, C)
        im = {
            "hsT": hst,
            "wq": weights[0], "wk": weights[1], "wv": weights[2],
            "wkg": weights[3], "wvg": weights[4], "wqg": weights[5],
            "masks": mk,
            "consts": consts,
        }
        if with_bias:
            im["biasrow"] = brow
        in_maps.append(im)
    return in_maps, with_bias


def kernel(hidden_states, Wq, bq, Wk, bk, Wv, bv, Wqg, bqg, Wkg, bkg,
           Wvg, bvg):
    hs = np.asarray(hidden_states, np.float32).reshape(S, H)
    weights = [np.ascontiguousarray(np.asarray(w, np.float32))
               for w in (Wq, Wk, Wv, Wkg, Wvg, Wqg)]
    biases = [np.asarray(b, np.float32)
              for b in (bq, bk, bv, bkg, bvg, bqg)]
    in_maps, with_bias = _host_inputs(hs, weights, biases)

    if with_bias not in _PROG_CACHE:
        _PROG_CACHE[with_bias] = _build_program(with_bias)
    nc = _PROG_CACHE[with_bias]

    res = run_bass_kernel_spmd(nc, in_maps, list(range(NCORE)))

    out = np.empty((S, H), np.float32)
    for core in range(NCORE):
        out[TPC * core:TPC * (core + 1)] = res.results[core]["out"]
    out[:G] = res.results[0]["outg"]
    return out.reshape(1, S, H)


# revision 4
# speedup vs baseline: 1.1838x; 1.1838x over previous
"""Longformer sliding-window + global attention layer on 8 Trainium2 NeuronCores.

Sharding: sequence-parallel over the 4096 tokens (512 per core, all 12 heads).
Each core recomputes the k/v halo (256 tokens each side) and the 64 global
k/v tokens locally from zero-padded hsT input, so the program is uniform SPMD.
The global-query rows (first 64 tokens attend to everything) are computed as
flash-style partial sums over each core's 512 tokens and combined with an
on-device AllReduce; every core finalizes the identical 64 global rows.

Layout strategy (all matmuls float32r = TF32):
  - hsT [hidden, tokens] feeds projections in both orientations.
  - Banded attention uses transposed scores over the core's 8 extended-window
    key tiles (jx): scoresT[j, t] = kT-slice.T @ qT with t spanning both local
    chunks wherever the 768-wide chunk windows overlap (N=512 matmuls), exp
    (scale=1/8 folded in), 0/1 mask multiply on the band/col boundary regions,
    then PV with natural-v tiles carrying an appended ones-column so the
    softmax denominator falls out of the same PSUM accumulation. A PE
    transpose + per-partition reciprocal scale normalizes [t, d] output tiles.
"""
import numpy as np

import concourse.bacc as bacc
import concourse.mybir as mybir
import concourse.tile as tile
from concourse.bass_utils import run_bass_kernel_spmd

F32 = mybir.dt.float32
F32R = mybir.dt.float32r
Exp = mybir.ActivationFunctionType.Exp

S, H, NH, HD = 4096, 768, 12, 64
C = 256               # chunk / one-sided window
G = 64                # global tokens
NCORE = 8
TPC = S // NCORE      # 512 tokens per core
EXT = TPC + 2 * C     # 1024 ext window
COLS = EXT + G        # 1088 = ext | glob
KC = H // 128         # 6 hidden chunks
VW = 66               # per-head v block: 64 v | ones | pad
SCALE = 1.0 / 8.0     # 1/sqrt(HD)

# per key-tile jx: (t0, tn) of the query range its chunk windows cover,
# and the t-range needing the 0/1 mask multiply (None = no mask)
JX_T = {0: (0, 256), 1: (0, 256), 2: (0, 512), 3: (0, 512),
        4: (0, 512), 5: (0, 512), 6: (256, 512), 7: (256, 512)}
JX_MASK = {0: (0, 256), 1: (0, 256), 2: (0, 512), 3: (256, 512),
           4: (0, 256), 5: (0, 256), 6: (256, 512), 7: (256, 512)}
JX_ORDER = [2, 3, 4, 5, 0, 1, 6, 7]   # full-width PV first (PSUM start=True)
# packed col offset of each jx's mask region in the [128, 2304] masks input
JX_MOFF = {0: 0, 1: 256, 2: 512, 3: 1024, 4: 1280, 5: 1536, 6: 1792, 7: 2048}

_PROG_CACHE = {}


def _build_program(with_bias: bool):
    nc = bacc.Bacc("TRN2", target_bir_lowering=False, debug=False,
                   num_devices=NCORE)
    d_hsT = nc.declare_dram_parameter("hsT", [H, COLS], F32R, isOutput=False)
    d_w = {
        n: nc.declare_dram_parameter(n, [H, H], F32R, isOutput=False)
        for n in ("wq", "wk", "wv", "wkg", "wvg", "wqg")
    }
    d_masks = nc.declare_dram_parameter("masks", [128, 2304], F32R,
                                        isOutput=False)
    d_consts = nc.declare_dram_parameter("consts", [128, 152], F32R,
                                         isOutput=False)
    if with_bias:
        d_brow = nc.declare_dram_parameter("biasrow", [7, COLS], F32R,
                                           isOutput=False)
    d_out = nc.declare_dram_parameter("out", [TPC, H], F32, isOutput=True)
    d_outg = nc.declare_dram_parameter("outg", [G, H], F32, isOutput=True)

    with tile.TileContext(nc) as tc:
        with (
            tc.tile_pool(name="const", bufs=1) as const,
            tc.tile_pool(name="wfull", bufs=2) as wfull,
            tc.tile_pool(name="work", bufs=2) as work,
            tc.tile_pool(name="w2", bufs=3) as w2,
            tc.tile_pool(name="late", bufs=1) as late,
            tc.tile_pool(name="dram", bufs=2, space="DRAM") as dram,
            tc.tile_pool(name="psB", bufs=3, space="PSUM") as psB,
            tc.tile_pool(name="psO", bufs=2, space="PSUM") as psO,
            tc.tile_pool(name="psT", bufs=3, space="PSUM") as psT,
        ):
            # ---- resident loads (weights first: they gate the first matmuls) ----
            csb = const.tile([128, 152], F32R)
            nc.sync.dma_start(out=csb, in_=d_consts[:])
            ident = csb[:, 0:128]
            vpat = csb[:, 128:152]          # [128, 12*2] ones/zeros pattern

            hsb = late.tile([128, KC, COLS], F32R, tag="ph")
            nc.sync.dma_start(out=hsb,
                              in_=d_hsT.rearrange("(kc p) t -> p kc t", p=128))
            if with_bias:
                bsb = const.tile([7, COLS], F32R)
                nc.sync.dma_start(out=bsb, in_=d_brow[:])

            kT = const.tile([128, KC, COLS], F32R)    # [o, t] all heads
            qT = const.tile([128, KC, TPC], F32R)
            vE = const.tile([128, 9, NH * VW], F32R)  # natural v + ones cols
            kgT = const.tile([128, KC, TPC], F32R)
            vgN = const.tile([128, 4, NH * VW], F32R)
            qgT = const.tile([128, KC, G], F32R)
            qgn = const.tile([G, H], F32R)

            def load_w(name):
                t = wfull.tile([128, KC, H], F32R, tag="wfull")
                nc.sync.dma_start(
                    out=t, in_=d_w[name].rearrange("(kc p) o -> p kc o", p=128))
                return t

            def proj_T(dst, wsb, segs, bias_idx, dst_off):
                # dst[o, t] = W.T @ hsT cols
                for oc in range(KC):
                    for c0, cn in segs:
                        ps = psB.tile([128, 512], F32, tag="psB")
                        for kc in range(KC):
                            nc.tensor.matmul(
                                out=ps[:, 0:cn],
                                lhsT=wsb[:, kc, oc * 128:(oc + 1) * 128],
                                rhs=hsb[:, kc, c0:c0 + cn],
                                start=(kc == 0),
                                stop=(kc == KC - 1 and not with_bias),
                            )
                        if with_bias:
                            nc.tensor.matmul(
                                out=ps[:, 0:cn],
                                lhsT=bsb[1 + bias_idx:2 + bias_idx,
                                         oc * 128:(oc + 1) * 128],
                                rhs=bsb[0:1, 0:cn],
                                start=False, stop=True,
                            )
                        nc.vector.tensor_copy(
                            out=dst[:, oc, c0 - dst_off:c0 - dst_off + cn],
                            in_=ps[:, 0:cn])

            def proj_nat(dst, wsb, tts, tok_off, bias_idx):
                # dst[t, head-block] with 66-stride head blocks
                for ti, tt in enumerate(tts):
                    # ones/pad pattern for cols 64/65 of each 66-block
                    nc.sync.dma_start(
                        out=dst[:, ti, :].rearrange("p (h x) -> p h x", x=VW)[:, :, 64:66],
                        in_=vpat.rearrange("p (h x) -> p h x", x=2))
                    tok0 = tok_off + tt * 128
                    rows = 128 if tok0 + 128 <= COLS else COLS - tok0
                    for o0, on in ((0, 512), (512, 256)):
                        ps = psB.tile([128, 512], F32, tag="psB")
                        for kc in range(KC):
                            nc.tensor.matmul(
                                out=ps[:rows, 0:on],
                                lhsT=hsb[:, kc, tok0:tok0 + rows],
                                rhs=wsb[:, kc, o0:o0 + on],
                                start=(kc == 0),
                                stop=(kc == KC - 1 and not with_bias),
                            )
                        if with_bias:
                            nc.tensor.matmul(
                                out=ps[:rows, 0:on],
                                lhsT=bsb[0:1, 0:rows],
                                rhs=bsb[1 + bias_idx:2 + bias_idx, o0:o0 + on],
                                start=False, stop=True,
                            )
                        nc.vector.tensor_copy(
                            out=dst[:rows, ti, :].rearrange(
                                "p (h x) -> p h x", x=VW)[:, o0 // 64:(o0 + on) // 64, 0:64],
                            in_=ps[:rows, 0:on].rearrange("p (h x) -> p h x", x=64))

            # ---- projections feeding the global-row partials first ----
            w = load_w("wkg")
            proj_T(kgT, w, ((C, 512),), 3, C)
            w = load_w("wvg")
            proj_nat(vgN, w, (2, 3, 4, 5), 0, 4)
            w = load_w("wqg")
            # qg natural [G, H] then PE-transpose into qgT
            for o0, on in ((0, 512), (512, 256)):
                ps = psB.tile([128, 512], F32, tag="psB")
                for kc in range(KC):
                    nc.tensor.matmul(
                        out=ps[0:G, 0:on],
                        lhsT=hsb[:, kc, EXT:EXT + G],
                        rhs=w[:, kc, o0:o0 + on],
                        start=(kc == 0), stop=(kc == KC - 1 and not with_bias))
                if with_bias:
                    nc.tensor.matmul(
                        out=ps[0:G, 0:on], lhsT=bsb[0:1, 0:G],
                        rhs=bsb[6:7, o0:o0 + on], start=False, stop=True)
                nc.vector.tensor_copy(out=qgn[:, o0:o0 + on], in_=ps[0:G, 0:on])
            for oc in range(KC):
                pstr = psT.tile([128, G], F32R, tag="psT")
                nc.tensor.transpose(pstr, qgn[0:G, oc * 128:(oc + 1) * 128],
                                    ident[0:G, 0:G])
                nc.vector.tensor_copy(out=qgT[:, oc, :], in_=pstr)

            # ---- global-row partial attention over own 512 tokens ----
            partial = dram.tile([G, NH * VW], F32)
            reduced = dram.tile([G, NH * VW], F32)
            for h in range(NH):
                dd = 64 * (h % 2)
                pc = h // 2
                psg = psB.tile([G, 512], F32, tag="psB")
                nc.tensor.matmul(out=psg, lhsT=qgT[dd:dd + 64, pc, :],
                                 rhs=kgT[dd:dd + 64, pc, :],
                                 start=True, stop=True)
                pg = work.tile([G, 512], F32R, tag="pg")
                nc.scalar.activation(out=pg, in_=psg, func=Exp, scale=SCALE)
                roff = min(VW * h, NH * VW - 288)
                boff = VW * h - roff
                pspv = psB.tile([G, 288], F32, tag="psB")
                for kt in range(4):
                    pstr = psT.tile([128, G], F32R, tag="psT")
                    nc.tensor.transpose(pstr, pg[:, kt * 128:(kt + 1) * 128],
                                        ident[0:G, 0:G])
                    pgt = w2.tile([128, G], F32R, tag="pgt")
                    nc.vector.tensor_copy(out=pgt, in_=pstr)
                    nc.tensor.matmul(out=pspv, lhsT=pgt,
                                     rhs=vgN[:, kt, roff:roff + 288],
                                     start=(kt == 0), stop=(kt == 3))
                part = w2.tile([G, VW], F32, tag="part")
                nc.vector.tensor_copy(out=part, in_=pspv[:, boff:boff + VW])
                nc.sync.dma_start(out=partial[:, h * VW:(h + 1) * VW], in_=part)

            nc.gpsimd.collective_compute(
                "AllReduce", mybir.AluOpType.add,
                replica_groups=[list(range(NCORE))],
                ins=[partial.opt()], outs=[reduced.opt()])

            # ---- banded + global-column attention (the bulk) ----
            msb = const.tile([128, 2304], F32R)
            nc.sync.dma_start(out=msb, in_=d_masks[:])
            w = load_w("wq")
            proj_T(qT, w, ((C, 512),), 0, C)
            w = load_w("wk")
            proj_T(kT, w, ((0, 512), (512, 320), (832, 256)), 1, 0)
            w = load_w("wv")
            proj_nat(vE, w, (0, 1, 2, 3, 4, 5, 6, 7, 8), 0, 2)

            osb = late.tile([128, 4, H], F32, tag="ph")
            for h in range(NH):
                dd = 64 * (h % 2)
                pc = h // 2
                pso = psO.tile([VW, TPC], F32, tag="psO")
                for jx in JX_ORDER:
                    t0, tn = JX_T[jx]
                    w_ = tn - t0
                    pss = psB.tile([128, 512], F32, tag="psB")
                    nc.tensor.matmul(
                        out=pss[:, 0:w_],
                        lhsT=kT[dd:dd + 64, pc, 128 * jx:128 * (jx + 1)],
                        rhs=qT[dd:dd + 64, pc, t0:tn],
                        start=True, stop=True)
                    ex = work.tile([128, 512], F32R, tag="ex", bufs=6)
                    nc.scalar.activation(out=ex[:, 0:w_], in_=pss[:, 0:w_],
                                         func=Exp, scale=SCALE)
                    m0, m1 = JX_MASK[jx]
                    mo = JX_MOFF[jx]
                    nc.vector.tensor_mul(
                        ex[:, m0 - t0:m1 - t0], ex[:, m0 - t0:m1 - t0],
                        msb[:, mo:mo + m1 - m0])
                    nc.tensor.matmul(
                        out=pso[:, t0:tn], lhsT=vE[:, jx, VW * h:VW * (h + 1)],
                        rhs=ex[:, 0:w_], start=(jx == 2), stop=False)
                # global-key columns, joint softmax
                pss = psB.tile([128, 512], F32, tag="psB")
                nc.tensor.matmul(
                    out=pss[0:G, :], lhsT=kT[dd:dd + 64, pc, EXT:EXT + G],
                    rhs=qT[dd:dd + 64, pc, :],
                    start=True, stop=True)
                exg = work.tile([G, TPC], F32R, tag="exg")
                nc.scalar.activation(out=exg, in_=pss[0:G, :], func=Exp,
                                     scale=SCALE)
                nc.tensor.matmul(
                    out=pso, lhsT=vE[0:G, 8, VW * h:VW * (h + 1)],
                    rhs=exg, start=False, stop=True)
                ot = w2.tile([VW, TPC], F32R, tag="ot")
                nc.vector.tensor_copy(out=ot, in_=pso)
                for tt in range(4):
                    pstr = psT.tile([128, VW], F32R, tag="psT")
                    nc.tensor.transpose(pstr, ot[:, tt * 128:(tt + 1) * 128],
                                        ident[0:VW, 0:VW])
                    rec = work.tile([128, 1], F32, tag="rec", bufs=4)
                    nc.vector.reciprocal(out=rec, in_=pstr[:, 64:65])
                    nc.vector.tensor_scalar_mul(
                        osb[:, tt, 64 * h:64 * (h + 1)],
                        in0=pstr[:, 0:64], scalar1=rec)
            for i4 in range(4):
                nc.sync.dma_start(out=d_out[128 * i4:128 * (i4 + 1), :],
                                  in_=osb[:, i4, :])

            # ---- finalize global rows from the AllReduced partials ----
            red = late.tile([G, NH * VW], F32, tag="red")
            nc.sync.dma_start(out=red, in_=reduced)
            ogsb = late.tile([G, H], F32, tag="ogsb")
            for h in range(NH):
                rec = work.tile([G, 1], F32, tag="recg")
                nc.vector.reciprocal(out=rec, in_=red[:, h * VW + 64:h * VW + 65])
                nc.vector.tensor_scalar_mul(
                    ogsb[:, h * 64:(h + 1) * 64],
                    in0=red[:, h * VW:h * VW + 64], scalar1=rec)
            nc.sync.dma_start(out=d_outg[:], in_=ogsb)

    nc.compile()
    return nc


def _host_inputs(hs, weights, biases):
    """Build the 8 per-core input maps from full inputs."""
    hsT = np.ascontiguousarray(hs.T)               # [H, S]
    ident = np.eye(128, dtype=np.float32)
    vpat = np.zeros((128, 24), np.float32)
    vpat[:, 0::2] = 1.0
    consts = np.concatenate([ident, vpat], axis=1)  # [128, 152]

    with_bias = any(np.any(b) for b in biases)
    if with_bias:
        brow = np.zeros((7, COLS), np.float32)
        brow[0, :] = 1.0
        for i, b in enumerate(biases):
            brow[1 + i, :H] = b
    # masks per core: [8 jx, 128, 512] over the two local chunks
    pp = np.arange(128)[:, None]                    # key pos within jx tile
    ii = np.arange(C)[None, :]                      # query pos within chunk
    in_maps = []
    for core in range(NCORE):
        hst = np.zeros((H, COLS), np.float32)
        lo = TPC * core - C
        hi = TPC * core + TPC + C
        clo, chi = max(lo, 0), min(hi, S)
        hst[:, clo - lo:chi - lo] = hsT[:, clo:chi]
        hst[:, EXT:] = hsT[:, :G]
        mk = np.ones((128, 2304), np.float32)
        for jx in range(8):
            m0, m1 = JX_MASK[jx]
            mo = JX_MOFF[jx]
            for cl in range(2):
                jt = jx - 2 * cl
                if not 0 <= jt <= 5:
                    continue
                t_lo = C * cl
                if t_lo < m0 or t_lo >= m1:
                    continue
                n = 2 * core + cl
                jj = 128 * jt + pp                  # strip pos within chunk
                ka = n * C - C + jj                 # absolute key pos
                valid = ((jj >= ii) & (jj <= ii + 2 * C)
                         & (ka >= G) & (ka < S))
                mk[:, mo + t_lo - m0:mo + t_lo - m0 + C] = valid
        im = {
            "hsT": hst,
            "wq": weights[0], "wk": weights[1], "wv": weights[2],
            "wkg": weights[3], "wvg": weights[4], "wqg": weights[5],
            "masks": mk,
            "consts": consts,
        }
        if with_bias:
            im["biasrow"] = brow
        in_maps.append(im)
    return in_maps, with_bias


def kernel(hidden_states, Wq, bq, Wk, bk, Wv, bv, Wqg, bqg, Wkg, bkg,
           Wvg, bvg):
    hs = np.asarray(hidden_states, np.float32).reshape(S, H)
    weights = [np.ascontiguousarray(np.asarray(w, np.float32))
               for w in (Wq, Wk, Wv, Wkg, Wvg, Wqg)]
    biases = [np.asarray(b, np.float32)
              for b in (bq, bk, bv, bkg, bvg, bqg)]
    in_maps, with_bias = _host_inputs(hs, weights, biases)

    if with_bias not in _PROG_CACHE:
        _PROG_CACHE[with_bias] = _build_program(with_bias)
    nc = _PROG_CACHE[with_bias]

    res = run_bass_kernel_spmd(nc, in_maps, list(range(NCORE)))

    out = np.empty((S, H), np.float32)
    for core in range(NCORE):
        out[TPC * core:TPC * (core + 1)] = res.results[core]["out"]
    out[:G] = res.results[0]["outg"]
    return out.reshape(1, S, H)
